# revision 1
# baseline (speedup 1.0000x reference)
"""CRF Viterbi decode on 8 Trainium2 NeuronCores (Bass/Tile).

Data-parallel: B=256 sharded as 32 samples/core; the [64,64] transition
matrix is replicated. Each core runs the max-plus forward scan with exact
first-index argmax backpointers (computed via the (best-cand)*BIG+p
min-reduce trick, bit-exact vs the fp32 reference), then a time-chunked
backtrace (4 chunks in parallel partition rows, 64-step speculative
warmup that is exact by survivor-path merging).

Self-contained: hardcodes B=256, S=2048, C=64.
"""
import os
import numpy as np

import concourse.bass as bass
import concourse.mybir as mybir
from concourse import bacc
from concourse.bass_utils import run_bass_kernel_spmd

PAD, BOS, EOS = 0, 1, 2
C = 64
CHI = 4
CLO = 16
BLOC = 32
NCORES = 8
BIG = 1e9
F32 = mybir.dt.float32
BF16 = mybir.dt.bfloat16
I32 = mybir.dt.int32

S_FULL = 2048
CH_FULL = 128
W_FULL = 64
NCHUNK_FULL = 4


_MASKIDX_OP = None
_DOT_OP = None
_LERP_OP = None


def _register_op(NAME, spec, subdim):
    from concourse import dve_ops
    from concourse.dve_spec import lower
    from concourse.dve_uop import DveOpSpec
    for op in dve_ops.OPS:
        if op.name == NAME:
            return op
    row = max(dve_ops._SUB_OPCODE_FOR_NAME.values()) + 1
    shas = {}
    for ver in ("v3", "v4"):
        tmp = DveOpSpec(name=NAME, opcode=row, uops=lower(spec, ver=ver),
                        rd1_en=True)
        shas[ver] = tmp.sha(ver)
    op = dve_ops.DveOp(NAME, spec, subdim=subdim, uops_sha=shas)
    dve_ops.OPS.append(op)
    dve_ops._SUB_OPCODE_FOR_NAME[NAME] = row
    dve_ops.CUSTOM_DVE_SPECS[NAME] = spec
    return op


def register_dot_add():
    """accum_out[q] = sum_k in0[q,k]*in1[q,k]; out = elementwise product."""
    global _DOT_OP
    if _DOT_OP is None:
        from concourse.dve_spec import Spec, Src0, Src1, AluOp

        def ref(in0, in1, s0, s1, imm2):
            b = (np.asarray(in0, np.float32) * np.asarray(in1, np.float32))
            return b, b.reshape(b.shape[0], -1).sum(-1, keepdims=True)

        _DOT_OP = _register_op("DOT_ADD_ANT",
                               Spec(body=Src0 * Src1, accum=AluOp.ADD,
                                    reference=ref), False)
    return _DOT_OP


def register_lerp():
    """out = (in0 - in1) * s0 + in1   (per-partition scalar s0)."""
    global _LERP_OP
    if _LERP_OP is None:
        from concourse.dve_spec import Spec, Src0, Src1, C0

        def ref(in0, in1, s0, s1, imm2):
            sc = np.asarray(s0, np.float32).reshape(-1, 1) if np.ndim(s0) else np.float32(s0)
            return ((np.asarray(in0, np.float32) - np.asarray(in1, np.float32)) * sc
                    + np.asarray(in1, np.float32)).astype(np.float32)

        _LERP_OP = _register_op("LERP_ANT",
                                Spec(body=(Src0 - Src1) * C0 + Src1,
                                     reference=ref), False)
    return _LERP_OP


def register_cand_maskidx():
    """Fused DVE pass: out[q, p, c_lo] = (in1 - in0) * s0 + p (p = page idx)."""
    global _MASKIDX_OP
    if _MASKIDX_OP is not None:
        return _MASKIDX_OP
    from concourse import dve_ops
    from concourse.dve_spec import Spec, Src0, Src1, C0, Zero, One, PageIdx, lower
    from concourse.dve_uop import DveOpSpec

    NAME = "CAND_MASKIDX_ANT"
    for op in dve_ops.OPS:
        if op.name == NAME:
            _MASKIDX_OP = op
            return op
    from concourse.dve_spec import Idx, C1
    body = (Src1 - Src0) * C0 + Idx - PageIdx(Zero, C1)

    def ref(in0, in1, s0, s1, imm2):
        sc = np.asarray(s0, np.float32).reshape(-1, 1, 1) if np.ndim(s0) else np.float32(s0)
        x = (np.asarray(in1, np.float32) - np.asarray(in0, np.float32)) * sc
        P, Spg, N = in0.shape
        within = np.broadcast_to(np.arange(N, dtype=np.float32), (Spg, N))
        return (x + within[None]).astype(np.float32)

    spec = Spec(body=body, reference=ref)
    row = max(dve_ops._SUB_OPCODE_FOR_NAME.values()) + 1
    shas = {}
    for ver in ("v3", "v4"):
        tmp = DveOpSpec(name=NAME, opcode=row, uops=lower(spec, ver=ver), rd1_en=True)
        shas[ver] = tmp.sha(ver)
    op = dve_ops.DveOp(NAME, spec, subdim=True, uops_sha=shas)
    dve_ops.OPS.append(op)
    dve_ops._SUB_OPCODE_FOR_NAME[NAME] = row
    dve_ops.CUSTOM_DVE_SPECS[NAME] = spec
    _MASKIDX_OP = op
    return op


def reap(ap, dims, extra_offset=0):
    """Rebuild free dims of an AP: dims = [[stride, num], ...] (elements)."""
    return bass.AP(ap.tensor, ap.offset + extra_offset, [ap.ap[0]] + dims)


def host_consts(T, mask_full, S, CW, W):
    """Per-core constant tensors. mask_full: [32, S] this core's mask."""
    q = np.arange(128)
    chi = q // 32
    out = {}
    trep = np.zeros((128, C * CLO), np.float32)
    for cl in range(CLO):
        for p in range(C):
            trep[:, cl * C + p] = T[p, 16 * chi + cl]
    out["trep"] = trep
    iotap = np.zeros((128, C * CLO), np.float32)
    for p in range(C):
        iotap[:, p * CLO:(p + 1) * CLO] = p
    out["iotap"] = iotap
    tbos = np.zeros((128, CLO), np.float32)
    teos = np.zeros((128, CLO), np.float32)
    for cl in range(CLO):
        tbos[:, cl] = T[BOS, 16 * chi + cl]
        teos[:, cl] = T[16 * chi + cl, EOS]
    out["tbos"] = tbos
    out["teos"] = teos
    out["iota64"] = np.broadcast_to(np.arange(C, dtype=np.float32), (128, C)).copy()
    sel = np.zeros((CHI, 128, 128), np.float32)
    for c in range(CHI):
        for m in range(128):
            sel[c, 32 * c + (m % 32), m] = 1.0
    out["sel"] = sel
    JTOT = CW + W
    k = q // 32
    b = q % 32
    mask2 = np.zeros((128, JTOT), np.float32)
    mask3 = np.zeros((128, CW), np.float32)
    for j in range(JTOT):
        t = k * CW + j
        valid = t <= S - 2
        mask2[:, j] = np.where(valid, mask_full[b, np.minimum(t + 1, S - 1)], 0.0)
    for j in range(CW):
        mask3[:, j] = mask_full[b, k * CW + j]
    out["mask2"] = mask2
    out["mask3"] = mask3
    return out


def build(nc, S, CH, W, NCHUNK, REPS=1):
    MIOP = register_cand_maskidx()
    DOTOP = register_dot_add()
    LERPOP = register_lerp()
    CW = S // NCHUNK
    JTOT = CW + W
    FREE = C * CLO

    demis = nc.dram_tensor("emis", [BLOC, S, C], F32, kind="ExternalInput")
    dmask = nc.dram_tensor("mask", [BLOC, S], F32, kind="ExternalInput")
    dmaski = nc.dram_tensor("maski", [BLOC, S], I32, kind="ExternalInput")
    dtrep = nc.dram_tensor("trep", [128, FREE], F32, kind="ExternalInput")
    diotap = nc.dram_tensor("iotap", [128, FREE], F32, kind="ExternalInput")
    dtbos = nc.dram_tensor("tbos", [128, CLO], F32, kind="ExternalInput")
    dteos = nc.dram_tensor("teos", [128, CLO], F32, kind="ExternalInput")
    diota64 = nc.dram_tensor("iota64", [128, C], F32, kind="ExternalInput")
    dsel = nc.dram_tensor("sel", [CHI, 128, 128], F32, kind="ExternalInput")
    dmask2 = nc.dram_tensor("mask2", [128, JTOT], F32, kind="ExternalInput")
    dmask3 = nc.dram_tensor("mask3", [128, CW], F32, kind="ExternalInput")
    dout = nc.dram_tensor("out", [BLOC, S], I32, kind="ExternalOutput")

    import concourse.tile as tile
    with tile.TileContext(nc) as tc:
        with tc.tile_pool(name="sbuf", bufs=1) as pool, \
             tc.tile_pool(name="psum", bufs=1, space="PSUM") as psum:
            trep = pool.tile([128, FREE], F32)
            iotap = pool.tile([128, FREE], F32)
            tbos = pool.tile([128, CLO], F32)
            teos = pool.tile([128, CLO], F32)
            iota64 = pool.tile([128, C], F32)
            selw = [pool.tile([128, 128], F32, name=f"sel{c}") for c in range(CHI)]
            mask2 = pool.tile([128, JTOT], F32)
            mask3 = pool.tile([128, CW], F32)
            em = [pool.tile([128, CH * CLO], F32, name=f"em{i}") for i in range(2)]
            mk = [pool.tile([128, CH], I32, name=f"mk{i}") for i in range(2)]
            cand = [pool.tile([128, FREE], F32, name=f"cand{i}") for i in range(2)]
            best = [pool.tile([128, CLO], F32, name=f"best{i}") for i in range(2)]
            masked = pool.tile([128, FREE], F32)
            nalpha = [pool.tile([128, CLO], F32, name=f"nal{i}") for i in range(2)]
            aown = pool.tile([128, CLO], F32)
            arep = [psum.tile([128, C], F32, name=f"ar{i}") for i in range(2)]
            bstg = [pool.tile([128, CH * CLO], BF16, name=f"bs{i}") for i in range(2)]
            bp2 = pool.tile([128, JTOT * C], BF16)
            pathc = pool.tile([128, JTOT + 1], F32)
            h = pool.tile([128, C], F32)
            scr64 = pool.tile([128, C], F32)
            ftile = pool.tile([128, C], F32)
            rmaxt = pool.tile([128, 1], F32)
            fd = pool.tile([128, C], F32)
            fin = pool.tile([128, CLO], F32)
            outi = pool.tile([128, CW], I32)

            AL = mybir.AluOpType
            X = mybir.AxisListType.X

            nc.vector.memset(bp2[:], 0.0)
            nc.sync.dma_start(trep[:], dtrep[:])
            nc.sync.dma_start(iotap[:], diotap[:])
            nc.sync.dma_start(tbos[:], dtbos[:])
            nc.sync.dma_start(teos[:], dteos[:])
            nc.sync.dma_start(iota64[:], diota64[:])
            for c in range(CHI):
                nc.sync.dma_start(selw[c][:], dsel[c])
            nc.sync.dma_start(mask2[:], dmask2[:])
            nc.sync.dma_start(mask3[:], dmask3[:])

            def load_chunk(i):
                buf = i % 2
                t0 = i * CH
                for chi in range(CHI):
                    nc.sync.dma_start(
                        em[buf][32 * chi:32 * chi + 32, :],
                        demis[0:BLOC, t0:t0 + CH, 16 * chi:16 * chi + 16])
                    nc.sync.dma_start(
                        mk[buf][32 * chi:32 * chi + 32, :],
                        dmaski[0:BLOC, t0:t0 + CH])

            def replicate(pb):
                for c in range(CHI):
                    nc.tensor.matmul(
                        arep[pb][:, 16 * c:16 * c + 16],
                        selw[c][:], aown[:], start=True, stop=True)

            for _rep in range(REPS):
                load_chunk(0)
                nc.vector.tensor_add(aown[:], em[0][:, 0:CLO], tbos[:])
                replicate(0)

                def reformat(ib, nb):
                    buf = ib % 2
                    tb0 = ib * CH
                    k = tb0 // CW
                    j0 = tb0 - k * CW
                    for chi in range(CHI):
                        src = bstg[buf][32 * chi:32 * chi + 32, 0:nb * CLO]
                        dst = reap(bp2[32 * k:32 * k + 32, :], [[C, nb], [1, CLO]],
                                   extra_offset=j0 * C + CLO * chi)
                        nc.sync.dma_start(dst, src.rearrange("q (a b) -> q a b", a=nb))
                    if j0 == 0 and k >= 1:
                        ndup = min(nb, W)
                        for chi in range(CHI):
                            src = bstg[buf][32 * chi:32 * chi + 32, 0:ndup * CLO]
                            dst = reap(bp2[32 * (k - 1):32 * k, :], [[C, ndup], [1, CLO]],
                                       extra_offset=CW * C + CLO * chi)
                            nc.sync.dma_start(dst, src.rearrange("q (a b) -> q a b", a=ndup))

                for t in range(1, S):
                    i = t // CH
                    jin = t % CH
                    if jin == 0:
                        load_chunk(i)
                    pb = t % 2
                    tb = t - 1
                    ib = tb // CH
                    jb = tb % CH
                    nc.vector.tensor_add(
                        cand[pb][:].rearrange("q (a b) -> q a b", a=CLO),
                        trep[:].rearrange("q (a b) -> q a b", a=CLO),
                        reap(arep[1 - pb][:], [[0, CLO], [1, C]]))
                    nc.vector.tensor_reduce(
                        best[pb][:], cand[pb][:].rearrange("q (a b) -> q a b", a=CLO),
                        axis=X, op=AL.max)
                    nc.vector.tensor_add(nalpha[pb][:], best[pb][:],
                                         em[i % 2][:, jin * CLO:(jin + 1) * CLO])
                    nc.vector.copy_predicated(
                        aown[:], reap(mk[i % 2][:, jin:jin + 1], [[0, CLO]]),
                        nalpha[pb][:])
                    replicate(pb)
                    nc.vector._custom_dve(
                        MIOP,
                        out=masked[:].rearrange("q (a b) -> q a b", a=CLO),
                        in0=cand[pb][:].rearrange("q (a b) -> q a b", a=CLO),
                        in1=reap(best[pb][:], [[1, CLO], [0, C]]),
                        s0=float(BIG), s1=float(C), imm2=0.0)
                    nc.vector.tensor_reduce(
                        bstg[ib % 2][:, jb * CLO:(jb + 1) * CLO],
                        masked[:].rearrange("q (a b) -> q a b", a=CLO), axis=X, op=AL.min)
                    if jb == CH - 1 or t == S - 1:
                        reformat(ib, jb + 1)

                nc.vector.tensor_add(fin[:], aown[:], teos[:])
                for chi in range(CHI):
                    nc.sync.dma_start(ftile[96:128, 16 * chi:16 * chi + 16],
                                      fin[32 * chi:32 * chi + 32, 0:CLO])
                nc.vector.tensor_reduce(rmaxt[96:128, :], ftile[96:128, :], axis=X, op=AL.max)
                nc.vector.tensor_tensor(fd[96:128, :],
                                        reap(rmaxt[96:128, :], [[0, C]]),
                                        ftile[96:128, :], op=AL.subtract)
                nc.vector.scalar_tensor_tensor(fd[96:128, :], in0=fd[96:128, :],
                                               scalar=float(BIG), in1=iota64[96:128, :],
                                               op0=AL.mult, op1=AL.add)
                nc.vector.tensor_reduce(pathc[96:128, JTOT:JTOT + 1], fd[96:128, :],
                                        axis=X, op=AL.min)
                for k in range(3):
                    nc.sync.dma_start(pathc[32 * k:32 * k + 32, JTOT:JTOT + 1],
                                      pathc[96:128, JTOT:JTOT + 1])
                nc.vector.tensor_scalar(h[:], iota64[:], pathc[:, JTOT:JTOT + 1],
                                        None, op0=AL.is_equal)

                for j in range(JTOT - 1, -1, -1):
                    nc.vector._custom_dve(
                        DOTOP, out=scr64[:], in0=bp2[:, j * C:(j + 1) * C],
                        in1=h[:], accum_out=rmaxt[:], s0=0.0, s1=0.0, imm2=0.0)
                    nc.vector._custom_dve(
                        LERPOP, out=pathc[:, j:j + 1], in0=rmaxt[:],
                        in1=pathc[:, j + 1:j + 2], s0=mask2[:, j:j + 1],
                        s1=0.0, imm2=0.0)
                    nc.vector.tensor_scalar(h[:], iota64[:], pathc[:, j:j + 1],
                                            None, op0=AL.is_equal)

                nc.vector.tensor_tensor(outi[:], pathc[:, 0:CW], mask3[:], op=AL.mult)
                for k in range(NCHUNK):
                    nc.sync.dma_start(dout[0:BLOC, k * CW:(k + 1) * CW],
                                      outi[32 * k:32 * k + 32, :])
    nc.compile()
    return nc


_CACHE = {}
LAST_RESULTS = None


def _get_nc():
    key = (S_FULL, CH_FULL, W_FULL, NCHUNK_FULL)
    if key not in _CACHE:
        nc = bacc.Bacc(None, target_bir_lowering=False)
        build(nc, *key)
        _CACHE[key] = nc
    return _CACHE[key]


def kernel(emissions, mask, transitions):
    global LAST_RESULTS
    emissions = np.ascontiguousarray(emissions, dtype=np.float32)
    mask = np.ascontiguousarray(mask, dtype=np.float32)
    transitions = np.ascontiguousarray(transitions, dtype=np.float32)
    B, S, C_ = emissions.shape
    assert (B, S, C_) == (256, S_FULL, 64)

    nc = _get_nc()
    CW = S_FULL // NCHUNK_FULL
    in_maps = []
    for core in range(NCORES):
        sl = slice(core * BLOC, (core + 1) * BLOC)
        m = {"emis": emissions[sl], "mask": mask[sl],
             "maski": np.ascontiguousarray(mask[sl].astype(np.int32))}
        m.update(host_consts(transitions, mask[sl], S_FULL, CW, W_FULL))
        in_maps.append(m)

    trace = bool(int(os.environ.get("CRF_TRACE", "0")))
    res = run_bass_kernel_spmd(nc, in_maps, list(range(NCORES)), trace=trace)
    LAST_RESULTS = res
    out = np.concatenate([res.results[c]["out"] for c in range(NCORES)], axis=0)
    return out.astype(np.int64)



# revision 13
# speedup vs baseline: 1.4322x; 1.4322x over previous
"""CRF Viterbi decode on 8 Trainium2 NeuronCores (Bass/Tile).

Data-parallel: B=256 sharded as 32 samples/core; the [64,64] transition
matrix is replicated. Each core runs the max-plus forward scan with exact
first-index argmax backpointers (computed via the (best-cand)*BIG+p
min-reduce trick, bit-exact vs the fp32 reference), then a time-chunked
backtrace (4 chunks in parallel partition rows, 64-step speculative
warmup that is exact by survivor-path merging).

Self-contained: hardcodes B=256, S=2048, C=64.
"""
import os
import numpy as np

import concourse.bass as bass
import concourse.mybir as mybir
from concourse import bacc
from concourse.bass_utils import run_bass_kernel_spmd

PAD, BOS, EOS = 0, 1, 2
C = 64
CHI = 4
CLO = 16
BLOC = 32
NCORES = 8
BIG = 1e9
F32 = mybir.dt.float32
BF16 = mybir.dt.bfloat16
I32 = mybir.dt.int32

S_FULL = 2048
CH_FULL = 128
W_FULL = 64
NCHUNK_FULL = 4


_MASKIDX_OP = None
_DOT_OP = None
_LERP_OP = None


def _register_op(NAME, spec, subdim):
    from concourse import dve_ops
    from concourse.dve_spec import lower
    from concourse.dve_uop import DveOpSpec
    for op in dve_ops.OPS:
        if op.name == NAME:
            return op
    row = max(dve_ops._SUB_OPCODE_FOR_NAME.values()) + 1
    shas = {}
    for ver in ("v3", "v4"):
        tmp = DveOpSpec(name=NAME, opcode=row, uops=lower(spec, ver=ver),
                        rd1_en=True)
        shas[ver] = tmp.sha(ver)
    op = dve_ops.DveOp(NAME, spec, subdim=subdim, uops_sha=shas)
    dve_ops.OPS.append(op)
    dve_ops._SUB_OPCODE_FOR_NAME[NAME] = row
    dve_ops.CUSTOM_DVE_SPECS[NAME] = spec
    return op


def register_dot_add():
    """accum_out[q] = sum_k in0[q,k]*in1[q,k]; out = elementwise product."""
    global _DOT_OP
    if _DOT_OP is None:
        from concourse.dve_spec import Spec, Src0, Src1, AluOp

        def ref(in0, in1, s0, s1, imm2):
            b = (np.asarray(in0, np.float32) * np.asarray(in1, np.float32))
            return b, b.reshape(b.shape[0], -1).sum(-1, keepdims=True)

        _DOT_OP = _register_op("DOT_ADD_ANT",
                               Spec(body=Src0 * Src1, accum=AluOp.ADD,
                                    reference=ref), False)
    return _DOT_OP


def register_lerp():
    """out = (in0 - in1) * s0 + in1   (per-partition scalar s0)."""
    global _LERP_OP
    if _LERP_OP is None:
        from concourse.dve_spec import Spec, Src0, Src1, C0

        def ref(in0, in1, s0, s1, imm2):
            sc = np.asarray(s0, np.float32).reshape(-1, 1) if np.ndim(s0) else np.float32(s0)
            return ((np.asarray(in0, np.float32) - np.asarray(in1, np.float32)) * sc
                    + np.asarray(in1, np.float32)).astype(np.float32)

        _LERP_OP = _register_op("LERP_ANT",
                                Spec(body=(Src0 - Src1) * C0 + Src1,
                                     reference=ref), False)
    return _LERP_OP


def register_cand_maskidx():
    """Fused DVE pass: out[q, p, c_lo] = (in1 - in0) * s0 + p (p = page idx)."""
    global _MASKIDX_OP
    if _MASKIDX_OP is not None:
        return _MASKIDX_OP
    from concourse import dve_ops
    from concourse.dve_spec import Spec, Src0, Src1, C0, Zero, One, PageIdx, lower
    from concourse.dve_uop import DveOpSpec

    NAME = "CAND_MASKIDX_ANT"
    for op in dve_ops.OPS:
        if op.name == NAME:
            _MASKIDX_OP = op
            return op
    from concourse.dve_spec import Idx, C1
    body = (Src1 - Src0) * C0 + Idx - PageIdx(Zero, C1)

    def ref(in0, in1, s0, s1, imm2):
        sc = np.asarray(s0, np.float32).reshape(-1, 1, 1) if np.ndim(s0) else np.float32(s0)
        x = (np.asarray(in1, np.float32) - np.asarray(in0, np.float32)) * sc
        P, Spg, N = in0.shape
        within = np.broadcast_to(np.arange(N, dtype=np.float32), (Spg, N))
        return (x + within[None]).astype(np.float32)

    spec = Spec(body=body, reference=ref)
    row = max(dve_ops._SUB_OPCODE_FOR_NAME.values()) + 1
    shas = {}
    for ver in ("v3", "v4"):
        tmp = DveOpSpec(name=NAME, opcode=row, uops=lower(spec, ver=ver), rd1_en=True)
        shas[ver] = tmp.sha(ver)
    op = dve_ops.DveOp(NAME, spec, subdim=True, uops_sha=shas)
    dve_ops.OPS.append(op)
    dve_ops._SUB_OPCODE_FOR_NAME[NAME] = row
    dve_ops.CUSTOM_DVE_SPECS[NAME] = spec
    _MASKIDX_OP = op
    return op


_PPOPS = {}


def register_ppscan(NAME, scan_op):
    """Hand-built per-page (subdim) scan op: blk0 = binop(Src0, Src1), blk1 =
    scan register (reset at each SUB_DIM boundary). For PPMAXADD_ANT:
    out[k] = running per-page max of (Src0+Src1), written every element.
    For PPLTCNT_ANT: decimated out[page] = sum over page of (Src0 < Src1)
    = index of first element reaching the page max when Src0 is a running
    max stream and Src1 its page-final value."""
    global _PPOPS
    if NAME in _PPOPS:
        return _PPOPS[NAME]
    from concourse import dve_ops
    from concourse.dve_spec import Spec, Src0, Src1
    from concourse.dve_uop import (
        UopConfig, AluOp as UAlu, AluInp, InpSel, Trigger, OutSel, OutPath,
        DveOpSpec, ENABLE,
    )
    for op in dve_ops.OPS:
        if op.name == NAME:
            _PPOPS[NAME] = op
            return op
    decimate = scan_op is UAlu.ADD

    def mk_uop(kind):
        u = UopConfig()
        u.enable_input(InpSel.SRC_0, 1)
        u.enable_input(InpSel.SRC_1, 2)
        u.require_inp0 = ENABLE
        u.require_inp1 = ENABLE
        dp = u.datapath_config
        if scan_op is UAlu.ADD:
            dp[0].enable_alu(UAlu.IS_LT, AluInp.PREV_DELAY_0, AluInp.PREV_DELAY_1)
        else:
            dp[0].enable_alu(UAlu.ADD, AluInp.PREV_DELAY_0, AluInp.PREV_DELAY_1)
        if kind == "steady":
            dp[1].enable_alu(scan_op, AluInp.CURR_ALU_OUT, AluInp.PREV_ALU_OUT)
        else:
            dp[1].enable_alu(UAlu.BYPASS, AluInp.PREV_ALU_OUT, AluInp.PREV_ALU_OUT)
        for k in range(2, 8):
            dp[k].pass_through_alu()
        u.enable_output(OutSel.ALU_OUT, OutPath.WR0_LO)
        if decimate:
            u.out_last_subdim_enable = ENABLE
        if kind == "steady":
            u.trigger = (Trigger.SRC_TENSOR_DONE, Trigger.SUB_DIM_DONE, Trigger.NONE)
            u.next_uop = (0, 2, 0)
        else:
            u.trigger = (Trigger.SRC_TENSOR_DONE, Trigger.SUB_DIM_DONE, Trigger.COUNT)
            u.next_uop = (0, 2, 1)
            u.repeat_count = 1
        return u

    uops = [mk_uop("entry"), mk_uop("steady"), mk_uop("boundary")]
    row = max(dve_ops._SUB_OPCODE_FOR_NAME.values()) + 1
    spec = Spec(body=Src0 + Src1,
                reference=lambda in0, in1, s0, s1, imm2: (
                    np.asarray(in0, np.float32) + np.asarray(in1, np.float32)))
    myspec = {ver: DveOpSpec(name=NAME, opcode=row, uops=uops, rd1_en=True)
              for ver in ("v3", "v4")}
    shas = {ver: myspec[ver].sha(ver) for ver in ("v3", "v4")}
    op = dve_ops.DveOp(NAME, spec, subdim=True, uops_sha=shas)
    dve_ops.OPS.append(op)
    dve_ops._SUB_OPCODE_FOR_NAME[NAME] = row
    dve_ops.CUSTOM_DVE_SPECS[NAME] = spec
    for ver in ("v3", "v4"):
        dve_ops._COMPILE_CACHE[(NAME, ver)] = myspec[ver]
    _PPOPS[NAME] = op
    return op


def reap(ap, dims, extra_offset=0):
    """Rebuild free dims of an AP: dims = [[stride, num], ...] (elements)."""
    return bass.AP(ap.tensor, ap.offset + extra_offset, [ap.ap[0]] + dims)


def host_consts(T, mask_full, S, CW, W):
    """Per-core constant tensors. mask_full: [32, S] this core's mask."""
    q = np.arange(128)
    chi = q // 32
    out = {}
    trep = np.zeros((128, C * CLO), np.float32)
    for cl in range(CLO):
        for p in range(C):
            trep[:, cl * C + p] = T[p, 16 * chi + cl]
    out["trep"] = trep
    iotap = np.zeros((128, C * CLO), np.float32)
    for p in range(C):
        iotap[:, p * CLO:(p + 1) * CLO] = p
    out["iotap"] = iotap
    tbos = np.zeros((128, CLO), np.float32)
    teos = np.zeros((128, CLO), np.float32)
    for cl in range(CLO):
        tbos[:, cl] = T[BOS, 16 * chi + cl]
        teos[:, cl] = T[16 * chi + cl, EOS]
    out["tbos"] = tbos
    out["teos"] = teos
    out["iota64"] = np.broadcast_to(np.arange(C, dtype=np.float32), (128, C)).copy()
    sel = np.zeros((CHI, 128, 128), np.float32)
    for c in range(CHI):
        for m in range(128):
            sel[c, 32 * c + (m % 32), m] = 1.0
    out["sel"] = sel
    # sel2[k, m] = 1 iff k%32 == m%32: one matmul gathers each sample's full
    # alpha from the zero-padded staged aown64 (off-diagonal blocks are 0).
    sel2 = np.zeros((128, 128), np.float32)
    for k in range(128):
        for m in range(128):
            if k % 32 == m % 32:
                sel2[k, m] = 1.0
    out["sel2"] = sel2
    # iotapg[:, cl*64 + p] = p for the gpsimd 3-op maskidx on low pages
    NLO = 8
    iotapg = np.zeros((128, NLO * C), np.float32)
    for cl in range(NLO):
        iotapg[:, cl * C:(cl + 1) * C] = np.arange(C, dtype=np.float32)
    out["iotapg"] = iotapg
    JTOT = CW + W
    k = q // 32
    b = q % 32
    mask2 = np.zeros((128, JTOT), np.float32)
    mask3 = np.zeros((128, CW), np.float32)
    for j in range(JTOT):
        t = k * CW + j
        valid = t <= S - 2
        mask2[:, j] = np.where(valid, mask_full[b, np.minimum(t + 1, S - 1)], 0.0)
    for j in range(CW):
        mask3[:, j] = mask_full[b, k * CW + j]
    out["mask2"] = mask2
    out["mask3"] = mask3
    return out


def build(nc, S, CH, W, NCHUNK, REPS=1):
    MIOP = register_cand_maskidx()
    DOTOP = register_dot_add()
    LERPOP = register_lerp()
    from concourse.dve_uop import AluOp as UAlu
    PPMAX = register_ppscan("PPMAXADD_ANT", UAlu.MAX)
    PPLT = register_ppscan("PPLTCNT_ANT", UAlu.ADD)
    CW = S // NCHUNK
    JTOT = CW + W
    FREE = C * CLO

    demis = nc.dram_tensor("emis", [BLOC, S, C], F32, kind="ExternalInput")
    dmask = nc.dram_tensor("mask", [BLOC, S], F32, kind="ExternalInput")
    dmaski = nc.dram_tensor("maski", [BLOC, S], I32, kind="ExternalInput")
    dtrep = nc.dram_tensor("trep", [128, FREE], F32, kind="ExternalInput")
    diotap = nc.dram_tensor("iotap", [128, FREE], F32, kind="ExternalInput")
    dtbos = nc.dram_tensor("tbos", [128, CLO], F32, kind="ExternalInput")
    dteos = nc.dram_tensor("teos", [128, CLO], F32, kind="ExternalInput")
    diota64 = nc.dram_tensor("iota64", [128, C], F32, kind="ExternalInput")
    dsel = nc.dram_tensor("sel", [CHI, 128, 128], F32, kind="ExternalInput")
    dsel2 = nc.dram_tensor("sel2", [128, 128], F32, kind="ExternalInput")
    diotapg = nc.dram_tensor("iotapg", [128, 8 * C], F32, kind="ExternalInput")
    dmask2 = nc.dram_tensor("mask2", [128, JTOT], F32, kind="ExternalInput")
    dmask3 = nc.dram_tensor("mask3", [128, CW], F32, kind="ExternalInput")
    dout = nc.dram_tensor("out", [BLOC, S], I32, kind="ExternalOutput")

    import concourse.tile as tile
    with tile.TileContext(nc) as tc:
        with tc.tile_pool(name="sbuf", bufs=1) as pool, \
             tc.tile_pool(name="psum", bufs=1, space="PSUM") as psum:
            trep = pool.tile([128, FREE], F32)
            iotap = pool.tile([128, FREE], F32)
            tbos = pool.tile([128, CLO], F32)
            teos = pool.tile([128, CLO], F32)
            iota64 = pool.tile([128, C], F32)
            sel2 = pool.tile([128, 128], F32)
            iotapg = pool.tile([128, 8 * C], F32)
            bigt = pool.tile([128, 8 * C], F32)
            mask2 = pool.tile([128, JTOT], F32)
            mask3 = pool.tile([128, CW], F32)
            em = [pool.tile([128, CH * CLO], F32, name=f"em{i}") for i in range(2)]
            mk = [pool.tile([128, CH], I32, name=f"mk{i}") for i in range(2)]
            cand = [pool.tile([128, FREE], F32, name=f"cand{i}") for i in range(2)]
            best = [pool.tile([128, CLO], F32, name=f"best{i}") for i in range(2)]
            masked = [pool.tile([128, FREE], F32, name=f"msk{i}") for i in range(2)]
            t1g = pool.tile([128, 8 * C], F32)
            nalpha = [pool.tile([128, CLO], F32, name=f"nal{i}") for i in range(2)]
            aown = pool.tile([128, CLO], F32)
            aown64 = pool.tile([128, C], F32)
            arep = [psum.tile([128, C], F32, name=f"ar{i}") for i in range(2)]
            bstg = [pool.tile([128, CH * CLO], BF16, name=f"bs{i}") for i in range(2)]
            bp2 = pool.tile([128, JTOT * C], BF16)
            pathc = pool.tile([128, JTOT + 1], F32)
            h = pool.tile([128, C], F32)
            scr64 = pool.tile([128, C], F32)
            ftile = pool.tile([128, C], F32)
            rmaxt = pool.tile([128, 1], F32)
            fd = pool.tile([128, C], F32)
            fin = pool.tile([128, CLO], F32)
            outi = pool.tile([128, CW], I32)

            AL = mybir.AluOpType
            X = mybir.AxisListType.X

            nc.vector.memset(bp2[:], 0.0)
            nc.vector.memset(aown64[:], 0.0)
            nc.vector.memset(bigt[:], float(BIG))
            nc.sync.dma_start(trep[:], dtrep[:])
            nc.sync.dma_start(iotap[:], diotap[:])
            nc.sync.dma_start(tbos[:], dtbos[:])
            nc.sync.dma_start(teos[:], dteos[:])
            nc.sync.dma_start(iota64[:], diota64[:])
            nc.sync.dma_start(sel2[:], dsel2[:])
            nc.sync.dma_start(iotapg[:], diotapg[:])
            nc.sync.dma_start(mask2[:], dmask2[:])
            nc.sync.dma_start(mask3[:], dmask3[:])

            def load_chunk(i):
                buf = i % 2
                t0 = i * CH
                for chi in range(CHI):
                    nc.sync.dma_start(
                        em[buf][32 * chi:32 * chi + 32, :],
                        demis[0:BLOC, t0:t0 + CH, 16 * chi:16 * chi + 16])
                    nc.sync.dma_start(
                        mk[buf][32 * chi:32 * chi + 32, :],
                        dmaski[0:BLOC, t0:t0 + CH])

            def replicate(pb):
                nc.tensor.matmul(arep[pb][:, 0:C], sel2[:], aown64[:],
                                 start=True, stop=True)

            for _rep in range(REPS):
                load_chunk(0)
                nc.vector.tensor_add(aown[:], em[0][:, 0:CLO], tbos[:])
                for c in range(CHI):
                    nc.vector.tensor_scalar(
                        aown64[32 * c:32 * c + 32, 16 * c:16 * c + 16],
                        aown[32 * c:32 * c + 32, :], 0.0, None, op0=AL.add)
                replicate(0)

                def reformat(ib, nb):
                    buf = ib % 2
                    tb0 = ib * CH
                    k = tb0 // CW
                    j0 = tb0 - k * CW
                    for chi in range(CHI):
                        src = bstg[buf][32 * chi:32 * chi + 32, 0:nb * CLO]
                        dst = reap(bp2[32 * k:32 * k + 32, :], [[C, nb], [1, CLO]],
                                   extra_offset=j0 * C + CLO * chi)
                        nc.sync.dma_start(dst, src.rearrange("q (a b) -> q a b", a=nb))
                    if j0 == 0 and k >= 1:
                        ndup = min(nb, W)
                        for chi in range(CHI):
                            src = bstg[buf][32 * chi:32 * chi + 32, 0:ndup * CLO]
                            dst = reap(bp2[32 * (k - 1):32 * k, :], [[C, ndup], [1, CLO]],
                                       extra_offset=CW * C + CLO * chi)
                            nc.sync.dma_start(dst, src.rearrange("q (a b) -> q a b", a=ndup))

                for t in range(1, S):
                    i = t // CH
                    jin = t % CH
                    if jin == 0:
                        load_chunk(i)
                    pb = t % 2
                    tb = t - 1
                    ib = tb // CH
                    jb = tb % CH
                    # fused: cand[pb] holds the per-page RUNNING max of
                    # trep + alpha; last element of each page = the max.
                    nc.vector._custom_dve(
                        PPMAX,
                        out=cand[pb][:].rearrange("q (a b) -> q a b", a=CLO),
                        in0=trep[:].rearrange("q (a b) -> q a b", a=CLO),
                        in1=reap(arep[1 - pb][:], [[0, CLO], [1, C]]),
                        s0=0.0, s1=0.0, imm2=0.0)
                    bestv = reap(cand[pb][:], [[C, CLO]], extra_offset=C - 1)
                    nc.vector.tensor_tensor(
                        nalpha[pb][:], bestv,
                        em[i % 2][:, jin * CLO:(jin + 1) * CLO], op=AL.add)
                    for c in range(CHI):
                        nc.vector.copy_predicated(
                            aown64[32 * c:32 * c + 32, 16 * c:16 * c + 16],
                            reap(mk[i % 2][32 * c:32 * c + 32, jin:jin + 1],
                                 [[0, CLO]]),
                            nalpha[pb][32 * c:32 * c + 32, :])
                    replicate(pb)
                    # backpointer = #elements with runmax < pagemax (exact
                    # first-argmax); per-page IS_LT sum, decimated write.
                    nc.vector._custom_dve(
                        PPLT,
                        out=bstg[ib % 2][:, jb * CLO:(jb + 1) * CLO],
                        in0=cand[pb][:].rearrange("q (a b) -> q a b", a=CLO),
                        in1=reap(cand[pb][:], [[C, CLO], [0, C]],
                                 extra_offset=C - 1),
                        s0=0.0, s1=0.0, imm2=0.0)
                    if jb == CH - 1 or t == S - 1:
                        reformat(ib, jb + 1)

                for c in range(CHI):
                    nc.vector.tensor_add(
                        fin[32 * c:32 * c + 32, :],
                        aown64[32 * c:32 * c + 32, 16 * c:16 * c + 16],
                        teos[32 * c:32 * c + 32, :])
                for chi in range(CHI):
                    nc.sync.dma_start(ftile[96:128, 16 * chi:16 * chi + 16],
                                      fin[32 * chi:32 * chi + 32, 0:CLO])
                nc.vector.tensor_reduce(rmaxt[96:128, :], ftile[96:128, :], axis=X, op=AL.max)
                nc.vector.tensor_tensor(fd[96:128, :],
                                        reap(rmaxt[96:128, :], [[0, C]]),
                                        ftile[96:128, :], op=AL.subtract)
                nc.vector.scalar_tensor_tensor(fd[96:128, :], in0=fd[96:128, :],
                                               scalar=float(BIG), in1=iota64[96:128, :],
                                               op0=AL.mult, op1=AL.add)
                nc.vector.tensor_reduce(pathc[96:128, JTOT:JTOT + 1], fd[96:128, :],
                                        axis=X, op=AL.min)
                for k in range(3):
                    nc.sync.dma_start(pathc[32 * k:32 * k + 32, JTOT:JTOT + 1],
                                      pathc[96:128, JTOT:JTOT + 1])
                nc.vector.tensor_scalar(h[:], iota64[:], pathc[:, JTOT:JTOT + 1],
                                        None, op0=AL.is_equal)

                for j in range(JTOT - 1, -1, -1):
                    nc.vector._custom_dve(
                        DOTOP, out=scr64[:], in0=bp2[:, j * C:(j + 1) * C],
                        in1=h[:], accum_out=rmaxt[:], s0=0.0, s1=0.0, imm2=0.0)
                    nc.vector._custom_dve(
                        LERPOP, out=pathc[:, j:j + 1], in0=rmaxt[:],
                        in1=pathc[:, j + 1:j + 2], s0=mask2[:, j:j + 1],
                        s1=0.0, imm2=0.0)
                    nc.vector.tensor_scalar(h[:], iota64[:], pathc[:, j:j + 1],
                                            None, op0=AL.is_equal)

                nc.vector.tensor_tensor(outi[:], pathc[:, 0:CW], mask3[:], op=AL.mult)
                for k in range(NCHUNK):
                    nc.sync.dma_start(dout[0:BLOC, k * CW:(k + 1) * CW],
                                      outi[32 * k:32 * k + 32, :])
    nc.compile()
    return nc


_CACHE = {}
LAST_RESULTS = None


def _get_nc():
    key = (S_FULL, CH_FULL, W_FULL, NCHUNK_FULL)
    if key not in _CACHE:
        nc = bacc.Bacc(None, target_bir_lowering=False)
        build(nc, *key)
        _CACHE[key] = nc
    return _CACHE[key]


def kernel(emissions, mask, transitions):
    global LAST_RESULTS
    emissions = np.ascontiguousarray(emissions, dtype=np.float32)
    mask = np.ascontiguousarray(mask, dtype=np.float32)
    transitions = np.ascontiguousarray(transitions, dtype=np.float32)
    B, S, C_ = emissions.shape
    assert (B, S, C_) == (256, S_FULL, 64)

    nc = _get_nc()
    CW = S_FULL // NCHUNK_FULL
    in_maps = []
    for core in range(NCORES):
        sl = slice(core * BLOC, (core + 1) * BLOC)
        m = {"emis": emissions[sl], "mask": mask[sl],
             "maski": np.ascontiguousarray(mask[sl].astype(np.int32))}
        m.update(host_consts(transitions, mask[sl], S_FULL, CW, W_FULL))
        in_maps.append(m)

    trace = bool(int(os.environ.get("CRF_TRACE", "0")))
    res = run_bass_kernel_spmd(nc, in_maps, list(range(NCORES)), trace=trace)
    LAST_RESULTS = res
    out = np.concatenate([res.results[c]["out"] for c in range(NCORES)], axis=0)
    return out.astype(np.int64)



# revision 20
# speedup vs baseline: 1.6381x; 1.1438x over previous
"""CRF Viterbi decode on 8 Trainium2 NeuronCores (Bass/Tile).

Data-parallel: B=256 sharded as 32 samples/core; the [64,64] transition
matrix is replicated. Each core runs the max-plus forward scan with exact
first-index argmax backpointers (computed via the (best-cand)*BIG+p
min-reduce trick, bit-exact vs the fp32 reference), then a time-chunked
backtrace (4 chunks in parallel partition rows, 64-step speculative
warmup that is exact by survivor-path merging).

Self-contained: hardcodes B=256, S=2048, C=64.
"""
import os
import numpy as np

import concourse.bass as bass
import concourse.mybir as mybir
from concourse import bacc
from concourse.bass_utils import run_bass_kernel_spmd

PAD, BOS, EOS = 0, 1, 2
C = 64
CHI = 4
CLO = 16
BLOC = 32
NCORES = 8
BIG = 1e9
F32 = mybir.dt.float32
BF16 = mybir.dt.bfloat16
I32 = mybir.dt.int32

S_FULL = 2048
CH_FULL = 128
W_FULL = 64
NCHUNK_FULL = 4


_MASKIDX_OP = None
_DOT_OP = None
_LERP_OP = None


def _register_op(NAME, spec, subdim):
    from concourse import dve_ops
    from concourse.dve_spec import lower
    from concourse.dve_uop import DveOpSpec
    for op in dve_ops.OPS:
        if op.name == NAME:
            return op
    row = max(dve_ops._SUB_OPCODE_FOR_NAME.values()) + 1
    shas = {}
    for ver in ("v3", "v4"):
        tmp = DveOpSpec(name=NAME, opcode=row, uops=lower(spec, ver=ver),
                        rd1_en=True)
        shas[ver] = tmp.sha(ver)
    op = dve_ops.DveOp(NAME, spec, subdim=subdim, uops_sha=shas)
    dve_ops.OPS.append(op)
    dve_ops._SUB_OPCODE_FOR_NAME[NAME] = row
    dve_ops.CUSTOM_DVE_SPECS[NAME] = spec
    return op


def register_dot_add():
    """accum_out[q] = sum_k in0[q,k]*in1[q,k]; out = elementwise product."""
    global _DOT_OP
    if _DOT_OP is None:
        from concourse.dve_spec import Spec, Src0, Src1, AluOp

        def ref(in0, in1, s0, s1, imm2):
            b = (np.asarray(in0, np.float32) * np.asarray(in1, np.float32))
            return b, b.reshape(b.shape[0], -1).sum(-1, keepdims=True)

        _DOT_OP = _register_op("DOT_ADD_ANT",
                               Spec(body=Src0 * Src1, accum=AluOp.ADD,
                                    reference=ref), False)
    return _DOT_OP


def register_lerp():
    """out = (in0 - in1) * s0 + in1   (per-partition scalar s0)."""
    global _LERP_OP
    if _LERP_OP is None:
        from concourse.dve_spec import Spec, Src0, Src1, C0

        def ref(in0, in1, s0, s1, imm2):
            sc = np.asarray(s0, np.float32).reshape(-1, 1) if np.ndim(s0) else np.float32(s0)
            return ((np.asarray(in0, np.float32) - np.asarray(in1, np.float32)) * sc
                    + np.asarray(in1, np.float32)).astype(np.float32)

        _LERP_OP = _register_op("LERP_ANT",
                                Spec(body=(Src0 - Src1) * C0 + Src1,
                                     reference=ref), False)
    return _LERP_OP


def register_cand_maskidx():
    """Fused DVE pass: out[q, p, c_lo] = (in1 - in0) * s0 + p (p = page idx)."""
    global _MASKIDX_OP
    if _MASKIDX_OP is not None:
        return _MASKIDX_OP
    from concourse import dve_ops
    from concourse.dve_spec import Spec, Src0, Src1, C0, Zero, One, PageIdx, lower
    from concourse.dve_uop import DveOpSpec

    NAME = "CAND_MASKIDX_ANT"
    for op in dve_ops.OPS:
        if op.name == NAME:
            _MASKIDX_OP = op
            return op
    from concourse.dve_spec import Idx, C1
    body = (Src1 - Src0) * C0 + Idx - PageIdx(Zero, C1)

    def ref(in0, in1, s0, s1, imm2):
        sc = np.asarray(s0, np.float32).reshape(-1, 1, 1) if np.ndim(s0) else np.float32(s0)
        x = (np.asarray(in1, np.float32) - np.asarray(in0, np.float32)) * sc
        P, Spg, N = in0.shape
        within = np.broadcast_to(np.arange(N, dtype=np.float32), (Spg, N))
        return (x + within[None]).astype(np.float32)

    spec = Spec(body=body, reference=ref)
    row = max(dve_ops._SUB_OPCODE_FOR_NAME.values()) + 1
    shas = {}
    for ver in ("v3", "v4"):
        tmp = DveOpSpec(name=NAME, opcode=row, uops=lower(spec, ver=ver), rd1_en=True)
        shas[ver] = tmp.sha(ver)
    op = dve_ops.DveOp(NAME, spec, subdim=True, uops_sha=shas)
    dve_ops.OPS.append(op)
    dve_ops._SUB_OPCODE_FOR_NAME[NAME] = row
    dve_ops.CUSTOM_DVE_SPECS[NAME] = spec
    _MASKIDX_OP = op
    return op


_PPOPS = {}


def register_ppscan(NAME, scan_op):
    """Hand-built per-page (subdim) scan op: blk0 = binop(Src0, Src1), blk1 =
    scan register (reset at each SUB_DIM boundary). For PPMAXADD_ANT:
    out[k] = running per-page max of (Src0+Src1), written every element.
    For PPLTCNT_ANT: decimated out[page] = sum over page of (Src0 < Src1)
    = index of first element reaching the page max when Src0 is a running
    max stream and Src1 its page-final value."""
    global _PPOPS
    if NAME in _PPOPS:
        return _PPOPS[NAME]
    from concourse import dve_ops
    from concourse.dve_spec import Spec, Src0, Src1
    from concourse.dve_uop import (
        UopConfig, AluOp as UAlu, AluInp, InpSel, Trigger, OutSel, OutPath,
        DveOpSpec, ENABLE,
    )
    for op in dve_ops.OPS:
        if op.name == NAME:
            _PPOPS[NAME] = op
            return op
    decimate = scan_op is UAlu.ADD

    def mk_uop(kind):
        u = UopConfig()
        u.enable_input(InpSel.SRC_0, 1)
        u.enable_input(InpSel.SRC_1, 2)
        u.require_inp0 = ENABLE
        u.require_inp1 = ENABLE
        dp = u.datapath_config
        if scan_op is UAlu.ADD:
            dp[0].enable_alu(UAlu.IS_LT, AluInp.PREV_DELAY_0, AluInp.PREV_DELAY_1)
        else:
            dp[0].enable_alu(UAlu.ADD, AluInp.PREV_DELAY_0, AluInp.PREV_DELAY_1)
        if kind == "steady":
            dp[1].enable_alu(scan_op, AluInp.CURR_ALU_OUT, AluInp.PREV_ALU_OUT)
        else:
            dp[1].enable_alu(UAlu.BYPASS, AluInp.PREV_ALU_OUT, AluInp.PREV_ALU_OUT)
        for k in range(2, 8):
            dp[k].pass_through_alu()
        u.enable_output(OutSel.ALU_OUT, OutPath.WR0_LO)
        if decimate:
            u.out_last_subdim_enable = ENABLE
        if kind == "steady":
            u.trigger = (Trigger.SRC_TENSOR_DONE, Trigger.SUB_DIM_DONE, Trigger.NONE)
            u.next_uop = (0, 2, 0)
        else:
            u.trigger = (Trigger.SRC_TENSOR_DONE, Trigger.SUB_DIM_DONE, Trigger.COUNT)
            u.next_uop = (0, 2, 1)
            u.repeat_count = 1
        return u

    uops = [mk_uop("entry"), mk_uop("steady"), mk_uop("boundary")]
    row = max(dve_ops._SUB_OPCODE_FOR_NAME.values()) + 1
    spec = Spec(body=Src0 + Src1,
                reference=lambda in0, in1, s0, s1, imm2: (
                    np.asarray(in0, np.float32) + np.asarray(in1, np.float32)))
    myspec = {ver: DveOpSpec(name=NAME, opcode=row, uops=uops, rd1_en=True)
              for ver in ("v3", "v4")}
    shas = {ver: myspec[ver].sha(ver) for ver in ("v3", "v4")}
    op = dve_ops.DveOp(NAME, spec, subdim=True, uops_sha=shas)
    dve_ops.OPS.append(op)
    dve_ops._SUB_OPCODE_FOR_NAME[NAME] = row
    dve_ops.CUSTOM_DVE_SPECS[NAME] = spec
    for ver in ("v3", "v4"):
        dve_ops._COMPILE_CACHE[(NAME, ver)] = myspec[ver]
    _PPOPS[NAME] = op
    return op


def reap(ap, dims, extra_offset=0):
    """Rebuild free dims of an AP: dims = [[stride, num], ...] (elements)."""
    return bass.AP(ap.tensor, ap.offset + extra_offset, [ap.ap[0]] + dims)


def host_consts(T, mask_full, S, CW, W):
    """Per-core constant tensors. mask_full: [32, S] this core's mask."""
    q = np.arange(128)
    chi = q // 32
    out = {}
    trep = np.zeros((128, C * CLO), np.float32)
    for cl in range(CLO):
        for p in range(C):
            trep[:, cl * C + p] = T[p, 16 * chi + cl]
    out["trep"] = trep
    iotap = np.zeros((128, C * CLO), np.float32)
    for p in range(C):
        iotap[:, p * CLO:(p + 1) * CLO] = p
    out["iotap"] = iotap
    tbos = np.zeros((128, CLO), np.float32)
    teos = np.zeros((128, CLO), np.float32)
    for cl in range(CLO):
        tbos[:, cl] = T[BOS, 16 * chi + cl]
        teos[:, cl] = T[16 * chi + cl, EOS]
    out["tbos"] = tbos
    out["teos"] = teos
    out["iota64"] = np.broadcast_to(np.arange(C, dtype=np.float32), (128, C)).copy()
    sel = np.zeros((CHI, 128, 128), np.float32)
    for c in range(CHI):
        for m in range(128):
            sel[c, 32 * c + (m % 32), m] = 1.0
    out["sel"] = sel
    # sel2[k, m] = 1 iff k%32 == m%32: one matmul gathers each sample's full
    # alpha from the zero-padded staged aown64 (off-diagonal blocks are 0).
    sel2 = np.zeros((128, 128), np.float32)
    for k in range(128):
        for m in range(128):
            if k % 32 == m % 32:
                sel2[k, m] = 1.0
    out["sel2"] = sel2
    # iotapg[:, cl*64 + p] = p for the gpsimd 3-op maskidx on low pages
    NLO = 8
    iotapg = np.zeros((128, NLO * C), np.float32)
    for cl in range(NLO):
        iotapg[:, cl * C:(cl + 1) * C] = np.arange(C, dtype=np.float32)
    out["iotapg"] = iotapg
    # diagpat[q, p] = 1 iff p is in quadrant q//32's 16-col diagonal block
    diagpat = np.zeros((128, C), np.float32)
    for qq in range(128):
        c0 = (qq // 32) * 16
        diagpat[qq, c0:c0 + 16] = 1.0
    out["diagpat"] = diagpat.astype(np.int32)
    JTOT = CW + W
    k = q // 32
    b = q % 32
    mask2 = np.zeros((128, JTOT), np.float32)
    mask3 = np.zeros((128, CW), np.float32)
    for j in range(JTOT):
        t = k * CW + j
        valid = t <= S - 2
        mask2[:, j] = np.where(valid, mask_full[b, np.minimum(t + 1, S - 1)], 0.0)
    for j in range(CW):
        mask3[:, j] = mask_full[b, k * CW + j]
    out["mask2"] = mask2
    out["mask3"] = mask3
    return out


def build(nc, S, CH, W, NCHUNK, REPS=1):
    MIOP = register_cand_maskidx()
    DOTOP = register_dot_add()
    LERPOP = register_lerp()
    from concourse.dve_uop import AluOp as UAlu
    PPMAX = register_ppscan("PPMAXADD_ANT", UAlu.MAX)
    PPLT = register_ppscan("PPLTCNT_ANT", UAlu.ADD)
    CW = S // NCHUNK
    JTOT = CW + W
    FREE = C * CLO

    demis = nc.dram_tensor("emis", [BLOC, S, C], F32, kind="ExternalInput")
    dmask = nc.dram_tensor("mask", [BLOC, S], F32, kind="ExternalInput")
    dmaski = nc.dram_tensor("maski", [BLOC, S], I32, kind="ExternalInput")
    dtrep = nc.dram_tensor("trep", [128, FREE], F32, kind="ExternalInput")
    diotap = nc.dram_tensor("iotap", [128, FREE], F32, kind="ExternalInput")
    dtbos = nc.dram_tensor("tbos", [128, CLO], F32, kind="ExternalInput")
    dteos = nc.dram_tensor("teos", [128, CLO], F32, kind="ExternalInput")
    diota64 = nc.dram_tensor("iota64", [128, C], F32, kind="ExternalInput")
    dsel = nc.dram_tensor("sel", [CHI, 128, 128], F32, kind="ExternalInput")
    dsel2 = nc.dram_tensor("sel2", [128, 128], F32, kind="ExternalInput")
    diotapg = nc.dram_tensor("iotapg", [128, 8 * C], F32, kind="ExternalInput")
    ddiagpat = nc.dram_tensor("diagpat", [128, C], I32, kind="ExternalInput")
    dmask2 = nc.dram_tensor("mask2", [128, JTOT], F32, kind="ExternalInput")
    dmask3 = nc.dram_tensor("mask3", [128, CW], F32, kind="ExternalInput")
    dout = nc.dram_tensor("out", [BLOC, S], I32, kind="ExternalOutput")

    import concourse.tile as tile
    with tile.TileContext(nc) as tc:
        with tc.tile_pool(name="sbuf", bufs=1) as pool, \
             tc.tile_pool(name="psum", bufs=1, space="PSUM") as psum:
            trep = pool.tile([128, FREE], F32)
            iotap = pool.tile([128, FREE], F32)
            tbos = pool.tile([128, CLO], F32)
            teos = pool.tile([128, CLO], F32)
            iota64 = pool.tile([128, C], F32)
            sel2 = pool.tile([128, 128], F32)
            iotapg = pool.tile([128, 8 * C], F32)
            bigt = pool.tile([128, 8 * C], F32)
            mask2 = pool.tile([128, JTOT], F32)
            mask3 = pool.tile([128, CW], F32)
            em = [pool.tile([128, CH * CLO], F32, name=f"em{i}") for i in range(2)]
            mk = [pool.tile([128, CH], I32, name=f"mk{i}") for i in range(2)]
            diagpat = pool.tile([128, C], I32)
            dm = [pool.tile([128, C], I32, name=f"dm{i}") for i in range(2)]
            cand = [pool.tile([128, FREE], F32, name=f"cand{i}") for i in range(2)]
            best = [pool.tile([128, CLO], F32, name=f"best{i}") for i in range(2)]
            masked = [pool.tile([128, FREE], F32, name=f"msk{i}") for i in range(2)]
            t1g = pool.tile([128, 8 * C], F32)
            nalpha = [pool.tile([128, CLO], F32, name=f"nal{i}") for i in range(2)]
            aown = pool.tile([128, CLO], F32)
            aown64 = pool.tile([128, C], F32)
            arep = [psum.tile([128, C], F32, name=f"ar{i}") for i in range(2)]
            bstg = [pool.tile([128, CH * CLO], BF16, name=f"bs{i}") for i in range(2)]
            bp2 = pool.tile([128, JTOT * C], BF16)
            pathc = pool.tile([128, JTOT + 1], F32)
            h = pool.tile([128, C], F32)
            scr64 = pool.tile([128, C], F32)
            ftile = pool.tile([128, C], F32)
            rmaxt = pool.tile([128, 1], F32)
            fd = pool.tile([128, C], F32)
            fin = pool.tile([128, CLO], F32)
            outi = pool.tile([128, CW], I32)

            AL = mybir.AluOpType
            X = mybir.AxisListType.X

            nc.vector.memset(bp2[:], 0.0)
            nc.vector.memset(aown64[:], 0.0)
            nc.vector.memset(bigt[:], float(BIG))
            nc.sync.dma_start(trep[:], dtrep[:])
            nc.sync.dma_start(iotap[:], diotap[:])
            nc.sync.dma_start(tbos[:], dtbos[:])
            nc.sync.dma_start(teos[:], dteos[:])
            nc.sync.dma_start(iota64[:], diota64[:])
            nc.sync.dma_start(sel2[:], dsel2[:])
            nc.sync.dma_start(iotapg[:], diotapg[:])
            nc.sync.dma_start(diagpat[:], ddiagpat[:])
            nc.sync.dma_start(mask2[:], dmask2[:])
            nc.sync.dma_start(mask3[:], dmask3[:])

            def load_chunk(i):
                buf = i % 2
                t0 = i * CH
                for chi in range(CHI):
                    nc.sync.dma_start(
                        em[buf][32 * chi:32 * chi + 32, :],
                        demis[0:BLOC, t0:t0 + CH, 16 * chi:16 * chi + 16])
                    nc.sync.dma_start(
                        mk[buf][32 * chi:32 * chi + 32, :],
                        dmaski[0:BLOC, t0:t0 + CH])

            def replicate(pb):
                nc.tensor.matmul(arep[pb][:, 0:C], sel2[:], aown64[:],
                                 start=True, stop=True)

            for _rep in range(REPS):
                load_chunk(0)
                nc.vector.tensor_add(aown[:], em[0][:, 0:CLO], tbos[:])
                for c in range(CHI):
                    nc.vector.tensor_scalar(
                        aown64[32 * c:32 * c + 32, 16 * c:16 * c + 16],
                        aown[32 * c:32 * c + 32, :], 0.0, None, op0=AL.add)
                replicate(0)

                def reformat(ib, nb):
                    buf = ib % 2
                    tb0 = ib * CH
                    k = tb0 // CW
                    j0 = tb0 - k * CW
                    for chi in range(CHI):
                        src = bstg[buf][32 * chi:32 * chi + 32, 0:nb * CLO]
                        dst = reap(bp2[32 * k:32 * k + 32, :], [[C, nb], [1, CLO]],
                                   extra_offset=j0 * C + CLO * chi)
                        nc.sync.dma_start(dst, src.rearrange("q (a b) -> q a b", a=nb))
                    if j0 == 0 and k >= 1:
                        ndup = min(nb, W)
                        for chi in range(CHI):
                            src = bstg[buf][32 * chi:32 * chi + 32, 0:ndup * CLO]
                            dst = reap(bp2[32 * (k - 1):32 * k, :], [[C, ndup], [1, CLO]],
                                       extra_offset=CW * C + CLO * chi)
                            nc.sync.dma_start(dst, src.rearrange("q (a b) -> q a b", a=ndup))

                for t in range(1, S):
                    i = t // CH
                    jin = t % CH
                    if jin == 0:
                        load_chunk(i)
                    pb = t % 2
                    tb = t - 1
                    ib = tb // CH
                    jb = tb % CH
                    # fused: cand[pb] holds the per-page RUNNING max of
                    # trep + alpha; last element of each page = the max.
                    nc.vector._custom_dve(
                        PPMAX,
                        out=cand[pb][:].rearrange("q (a b) -> q a b", a=CLO),
                        in0=trep[:].rearrange("q (a b) -> q a b", a=CLO),
                        in1=reap(arep[1 - pb][:], [[0, CLO], [1, C]]),
                        s0=0.0, s1=0.0, imm2=0.0)
                    bestv = reap(cand[pb][:], [[C, CLO]], extra_offset=C - 1)
                    nc.vector.tensor_tensor(
                        nalpha[pb][:], bestv,
                        em[i % 2][:, jin * CLO:(jin + 1) * CLO], op=AL.add)
                    # diag-gated float mask, built off-chain on gpsimd
                    nc.gpsimd.tensor_tensor(
                        dm[pb][:], reap(mk[i % 2][:, jin:jin + 1], [[0, C]]),
                        diagpat[:], op=AL.mult)
                    nc.vector.copy_predicated(
                        aown64[:].rearrange("q (a b) -> q a b", a=CHI),
                        dm[pb][:].rearrange("q (a b) -> q a b", a=CHI),
                        reap(nalpha[pb][:], [[0, CHI], [1, CLO]]))
                    replicate(pb)
                    # backpointer = #elements with runmax < pagemax (exact
                    # first-argmax); per-page IS_LT sum, decimated write.
                    nc.vector._custom_dve(
                        PPLT,
                        out=bstg[ib % 2][:, jb * CLO:(jb + 1) * CLO],
                        in0=cand[pb][:].rearrange("q (a b) -> q a b", a=CLO),
                        in1=reap(cand[pb][:], [[C, CLO], [0, C]],
                                 extra_offset=C - 1),
                        s0=0.0, s1=0.0, imm2=0.0)
                    if jb == CH - 1 or t == S - 1:
                        reformat(ib, jb + 1)

                for c in range(CHI):
                    nc.vector.tensor_add(
                        fin[32 * c:32 * c + 32, :],
                        aown64[32 * c:32 * c + 32, 16 * c:16 * c + 16],
                        teos[32 * c:32 * c + 32, :])
                for chi in range(CHI):
                    nc.sync.dma_start(ftile[96:128, 16 * chi:16 * chi + 16],
                                      fin[32 * chi:32 * chi + 32, 0:CLO])
                nc.vector.tensor_reduce(rmaxt[96:128, :], ftile[96:128, :], axis=X, op=AL.max)
                nc.vector.tensor_tensor(fd[96:128, :],
                                        reap(rmaxt[96:128, :], [[0, C]]),
                                        ftile[96:128, :], op=AL.subtract)
                nc.vector.scalar_tensor_tensor(fd[96:128, :], in0=fd[96:128, :],
                                               scalar=float(BIG), in1=iota64[96:128, :],
                                               op0=AL.mult, op1=AL.add)
                nc.vector.tensor_reduce(pathc[96:128, JTOT:JTOT + 1], fd[96:128, :],
                                        axis=X, op=AL.min)
                for k in range(3):
                    nc.sync.dma_start(pathc[32 * k:32 * k + 32, JTOT:JTOT + 1],
                                      pathc[96:128, JTOT:JTOT + 1])
                nc.vector.tensor_scalar(h[:], iota64[:], pathc[:, JTOT:JTOT + 1],
                                        None, op0=AL.is_equal)

                for j in range(JTOT - 1, -1, -1):
                    nc.vector._custom_dve(
                        DOTOP, out=scr64[:], in0=bp2[:, j * C:(j + 1) * C],
                        in1=h[:], accum_out=rmaxt[:], s0=0.0, s1=0.0, imm2=0.0)
                    nc.vector._custom_dve(
                        LERPOP, out=pathc[:, j:j + 1], in0=rmaxt[:],
                        in1=pathc[:, j + 1:j + 2], s0=mask2[:, j:j + 1],
                        s1=0.0, imm2=0.0)
                    nc.vector.tensor_scalar(h[:], iota64[:], pathc[:, j:j + 1],
                                            None, op0=AL.is_equal)

                nc.vector.tensor_tensor(outi[:], pathc[:, 0:CW], mask3[:], op=AL.mult)
                for k in range(NCHUNK):
                    nc.sync.dma_start(dout[0:BLOC, k * CW:(k + 1) * CW],
                                      outi[32 * k:32 * k + 32, :])
    nc.compile()
    return nc


_CACHE = {}
LAST_RESULTS = None


def _get_nc():
    key = (S_FULL, CH_FULL, W_FULL, NCHUNK_FULL)
    if key not in _CACHE:
        nc = bacc.Bacc(None, target_bir_lowering=False)
        build(nc, *key)
        _CACHE[key] = nc
    return _CACHE[key]


def kernel(emissions, mask, transitions):
    global LAST_RESULTS
    emissions = np.ascontiguousarray(emissions, dtype=np.float32)
    mask = np.ascontiguousarray(mask, dtype=np.float32)
    transitions = np.ascontiguousarray(transitions, dtype=np.float32)
    B, S, C_ = emissions.shape
    assert (B, S, C_) == (256, S_FULL, 64)

    nc = _get_nc()
    CW = S_FULL // NCHUNK_FULL
    in_maps = []
    for core in range(NCORES):
        sl = slice(core * BLOC, (core + 1) * BLOC)
        m = {"emis": emissions[sl], "mask": mask[sl],
             "maski": np.ascontiguousarray(mask[sl].astype(np.int32))}
        m.update(host_consts(transitions, mask[sl], S_FULL, CW, W_FULL))
        in_maps.append(m)

    trace = bool(int(os.environ.get("CRF_TRACE", "0")))
    res = run_bass_kernel_spmd(nc, in_maps, list(range(NCORES)), trace=trace)
    LAST_RESULTS = res
    out = np.concatenate([res.results[c]["out"] for c in range(NCORES)], axis=0)
    return out.astype(np.int64)

